# revision 19
# baseline (speedup 1.0000x reference)
"""GatedDirGCNConv on 8 Trainium2 NeuronCores — device-built tables + gathers.

Node-partitioned per the sharding hint.  Host only routes edges (argsort +
index packing) and ships x transposed in bf16 plus the small weights; the
device builds the four linear node-feature tables [U|TS|V|TD] with TensorE
into a DRAM scratch, then per 128-node window gathers per-edge rows with
indirect DMA (512B bf16 descriptors), runs the edge MLP + sigmoid scores +
message scaling, scatter-adds via one-hot matmuls in PSUM, normalizes by
degree, applies the gate MLP + fusion + residual, and writes the output
shard.
"""

import sys
import time
import numpy as np
import ml_dtypes
import concourse.bass as bass
import concourse.bacc as bacc
import concourse.mybir as mybir
import concourse.tile as tile
from concourse.bass_utils import run_bass_kernel_spmd

F32 = mybir.dt.float32
BF16 = mybir.dt.bfloat16
I32 = mybir.dt.int32
U16 = mybir.dt.uint16
FP8 = mybir.dt.float8e4
P = 128
ALU = mybir.AluOpType
ACTF = mybir.ActivationFunctionType
NPBF = ml_dtypes.bfloat16
NPF8 = ml_dtypes.float8_e4m3

N_NODES = 50000
NC = 8
VERBOSE = False


def _t(msg, t0):
    if VERBOSE:
        print("[v3] %-18s %.3fs" % (msg, time.time() - t0), file=sys.stderr)
    return time.time()


def _build(nwin, T, has_b_g1):
    nc = bacc.Bacc("TRN2", target_bir_lowering=False, debug=False, num_devices=8)
    NW = nwin * P
    WT = nwin * T
    NCH = (N_NODES + P - 1) // P  # table-build chunks

    xT = nc.dram_tensor("xT", [P, N_NODES], FP8, kind="ExternalInput")
    W4 = nc.dram_tensor("W4", [P, 4 * P], FP8, kind="ExternalInput")
    b4 = nc.dram_tensor("b4", [1, 4 * P], FP8, kind="ExternalInput")
    idx0 = nc.dram_tensor("idx0", [P, WT], I32, kind="ExternalInput")
    jdx0 = nc.dram_tensor("jdx0", [P, WT], I32, kind="ExternalInput")
    idx1 = nc.dram_tensor("idx1", [P, WT], I32, kind="ExternalInput")
    jdx1 = nc.dram_tensor("jdx1", [P, WT], I32, kind="ExternalInput")
    dl0 = nc.dram_tensor("dl0", [P, WT], BF16, kind="ExternalInput")
    dl1 = nc.dram_tensor("dl1", [P, WT], BF16, kind="ExternalInput")
    rc0 = nc.dram_tensor("rc0", [P, nwin], F32, kind="ExternalInput")
    rc1 = nc.dram_tensor("rc1", [P, nwin], F32, kind="ExternalInput")
    x_own = nc.dram_tensor("x_own", [NW, P], BF16, kind="ExternalInput")

    wg1a = nc.dram_tensor("wg1a", [P, P], F32, kind="ExternalInput")
    wg1b = nc.dram_tensor("wg1b", [P, P], F32, kind="ExternalInput")
    we2r = nc.dram_tensor("we2r", [P, P], BF16, kind="ExternalInput")
    wg2r = nc.dram_tensor("wg2r", [P, P], F32, kind="ExternalInput")
    iota = nc.dram_tensor("iota", [P, P], BF16, kind="ExternalInput")
    ident = nc.dram_tensor("ident", [P, P], F32, kind="ExternalInput")
    be2c = nc.dram_tensor("be2c", [P, 1], F32, kind="ExternalInput")
    bg2c = nc.dram_tensor("bg2c", [P, 1], F32, kind="ExternalInput")
    ones_row = nc.dram_tensor("ones_row", [1, P], F32, kind="ExternalInput")
    bg1r = nc.dram_tensor("bg1r", [1, P], F32, kind="ExternalInput") if has_b_g1 else None

    T4 = nc.dram_tensor("T4", [N_NODES, 4 * P], BF16, kind="Internal")
    out = nc.dram_tensor("out", [NW, P], BF16, kind="ExternalOutput")

    from contextlib import ExitStack
    with tile.TileContext(nc) as tc, ExitStack() as stk:
        cp = stk.enter_context(tc.tile_pool(name="consts", bufs=1))
        gp = stk.enter_context(tc.tile_pool(name="gate", bufs=2))
        hp = stk.enter_context(tc.tile_pool(name="hres", bufs=1))

        def ld(name, src, shape, dt=F32):
            t = cp.tile(shape, dt, tag=name)
            nc.sync.dma_start(out=t[:], in_=src[:])
            return t

        wg1a_t = ld("wg1a", wg1a, [P, P])
        wg1b_t = ld("wg1b", wg1b, [P, P])
        we2r_t = ld("we2r", we2r, [P, P], BF16)
        wg2r_t = ld("wg2r", wg2r, [P, P])
        iota_t = ld("iota", iota, [P, P], BF16)
        ident_t = ld("ident", ident, [P, P])
        be2c_t = ld("be2c", be2c, [P, 1])
        bg2c_t = ld("bg2c", bg2c, [P, 1])
        ones_t = ld("ones_row", ones_row, [1, P])
        bg1r_t = ld("bg1r", bg1r, [1, P]) if has_b_g1 else None

        idx_t = [ld("idx0", idx0, [P, WT], I32), ld("idx1", idx1, [P, WT], I32)]
        jdx_t = [ld("jdx0", jdx0, [P, WT], I32), ld("jdx1", jdx1, [P, WT], I32)]
        dl_t = [ld("dl0", dl0, [P, WT], BF16), ld("dl1", dl1, [P, WT], BF16)]
        rc_t = [ld("rc0", rc0, [P, nwin]), ld("rc1", rc1, [P, nwin])]

        h_in = hp.tile([P, NW], F32, tag="h_in")
        h_out = hp.tile([P, NW], F32, tag="h_out")

        # ---- phase 0: build node tables T4 = [U|TS|V|TD] on device ----
        with tc.tile_pool(name="xt", bufs=1) as xp, \
             tc.tile_pool(name="tbp", bufs=3) as tp, \
             tc.tile_pool(name="ps_b", bufs=4, space="PSUM") as pb:
            xT_sb = xp.tile([P, N_NODES], FP8, tag="xT")
            nc.sync.dma_start(out=xT_sb[:], in_=xT[:])
            W4_t = xp.tile([P, 4 * P], FP8, tag="W4")
            nc.sync.dma_start(out=W4_t[:], in_=W4[:])
            b4_t = xp.tile([1, 4 * P], FP8, tag="b4")
            nc.sync.dma_start(out=b4_t[:], in_=b4[:])
            ones1 = xp.tile([1, P], FP8, tag="ones1")
            nc.vector.memset(ones1[:], 1.0)
            for k in range(NCH):
                r0 = k * P
                pn = min(P, N_NODES - r0)
                ps = pb.tile([P, 4 * P], F32, tag="tps")
                nc.tensor.matmul(out=ps[:pn, :], lhsT=xT_sb[:, r0:r0 + pn],
                                 rhs=W4_t[:], start=True, stop=False)
                nc.tensor.matmul(out=ps[:pn, :], lhsT=ones1[:, :pn],
                                 rhs=b4_t[:], start=False, stop=True)
                tb = tp.tile([P, 4 * P], BF16, tag="tb")
                nc.vector.tensor_copy(tb[:pn, :], ps[:pn, :])
                nc.sync.dma_start(out=T4[r0:r0 + pn, :], in_=tb[:pn, :])

        tc.strict_bb_all_engine_barrier()

        # ---- phase 1: edge passes ----
        with tc.tile_pool(name="edge", bufs=2) as ep:
            for d, h_sb in enumerate((h_in, h_out)):
                goff = 0 if d == 0 else 2 * P   # gm: U|TS vs V|TD
                voff = 2 * P if d == 0 else 0   # vg: V vs U
                with tc.tile_pool(name="ps_e%d" % d, bufs=2, space="PSUM") as pp:
                    for w in range(nwin):
                        cols = bass.ts(w, T)
                        rows = bass.ts(w, P)
                        gm = ep.tile([P, T, 2 * P], BF16, tag="gm")
                        vg = ep.tile([P, T, P], BF16, tag="vg")
                        for t in range(T):
                            c1 = w * T + t
                            nc.gpsimd.indirect_dma_start(
                                out=gm[:, t, :], out_offset=None, in_=T4[:],
                                in_offset=bass.IndirectOffsetOnAxis(
                                    ap=idx_t[d][:, c1:c1 + 1], axis=0),
                                element_offset=goff)
                            nc.gpsimd.indirect_dma_start(
                                out=vg[:, t, :], out_offset=None, in_=T4[:],
                                in_offset=bass.IndirectOffsetOnAxis(
                                    ap=jdx_t[d][:, c1:c1 + 1], axis=0),
                                element_offset=voff)

                        pre = ep.tile([P, T, P], BF16, tag="pre")
                        nc.vector.tensor_add(out=pre[:], in0=gm[:, :, 0:P],
                                             in1=vg[:])
                        he = ep.tile([P, T, P], BF16, tag="he")
                        nc.scalar.activation(he[:], pre[:], ACTF.Relu)
                        sm = ep.tile([P, T, P], BF16, tag="sm")
                        nc.vector.tensor_tensor(
                            out=sm[:], in0=he[:],
                            in1=we2r_t[:].unsqueeze(1).to_broadcast([P, T, P]),
                            op=ALU.mult)
                        sp = ep.tile([P, T], F32, tag="sp")
                        nc.vector.tensor_reduce(
                            out=sp[:], in_=sm[:],
                            axis=mybir.AxisListType.X, op=ALU.add)
                        sc = ep.tile([P, T], BF16, tag="sc")
                        nc.scalar.activation(sc[:], sp[:], ACTF.Sigmoid,
                                             bias=be2c_t[:])
                        msg = ep.tile([P, T, P], BF16, tag="msg")
                        nc.vector.tensor_tensor(
                            out=msg[:], in0=gm[:, :, P:2 * P],
                            in1=sc[:].unsqueeze(2).to_broadcast([P, T, P]),
                            op=ALU.mult)
                        seg = ep.tile([P, T, P], BF16, tag="seg")
                        nc.vector.tensor_tensor(
                            out=seg[:],
                            in0=dl_t[d][:, cols].unsqueeze(2).to_broadcast(
                                [P, T, P]),
                            in1=iota_t[:].unsqueeze(1).to_broadcast([P, T, P]),
                            op=ALU.is_equal)
                        acc = pp.tile([P, P], F32, tag="acc")
                        for t in range(T):
                            nc.tensor.matmul(out=acc[:], lhsT=seg[:, t, :],
                                             rhs=msg[:, t, :],
                                             start=(t == 0), stop=(t == T - 1))
                        nc.vector.tensor_scalar_mul(
                            h_sb[:, rows], acc[:], rc_t[d][:, w:w + 1])

        # ---- phase 2: gate + fuse + residual ----
        with tc.tile_pool(name="ps_g", bufs=2, space="PSUM") as pp:
            for w in range(nwin):
                rows = bass.ts(w, P)
                hi = gp.tile([P, P], F32, tag="hi")
                nc.vector.tensor_copy(hi[:], h_in[:, rows])
                ho = gp.tile([P, P], F32, tag="ho")
                nc.vector.tensor_copy(ho[:], h_out[:, rows])
                t1 = pp.tile([P, P], F32, tag="t1")
                nc.tensor.transpose(out=t1[:], in_=hi[:], identity=ident_t[:])
                hiT = gp.tile([P, P], F32, tag="hiT")
                nc.scalar.copy(hiT[:], t1[:])
                t2 = pp.tile([P, P], F32, tag="t2")
                nc.tensor.transpose(out=t2[:], in_=ho[:], identity=ident_t[:])
                hoT = gp.tile([P, P], F32, tag="hoT")
                nc.scalar.copy(hoT[:], t2[:])
                hg_ps = pp.tile([P, P], F32, tag="hg_ps")
                if has_b_g1:
                    nc.tensor.matmul(out=hg_ps[:], lhsT=ones_t[:],
                                     rhs=bg1r_t[:], start=True, stop=False)
                    nc.tensor.matmul(out=hg_ps[:], lhsT=hiT[:], rhs=wg1a_t[:],
                                     start=False, stop=False)
                else:
                    nc.tensor.matmul(out=hg_ps[:], lhsT=hiT[:], rhs=wg1a_t[:],
                                     start=True, stop=False)
                nc.tensor.matmul(out=hg_ps[:], lhsT=hoT[:], rhs=wg1b_t[:],
                                 start=False, stop=True)
                hg = gp.tile([P, P], F32, tag="hg")
                nc.scalar.activation(hg[:], hg_ps[:], ACTF.Relu)
                gpre = gp.tile([P, 1], F32, tag="gpre")
                scr2 = gp.tile([P, P], F32, tag="scr2")
                nc.vector.tensor_tensor(out=scr2[:], in0=hg[:], in1=wg2r_t[:],
                                        op=ALU.mult)
                nc.vector.tensor_reduce(out=gpre[:], in_=scr2[:],
                                        axis=mybir.AxisListType.X, op=ALU.add)
                g = gp.tile([P, 1], F32, tag="g")
                nc.scalar.activation(g[:], gpre[:], ACTF.Sigmoid, bias=bg2c_t[:])
                diff = gp.tile([P, P], F32, tag="diff")
                nc.vector.tensor_tensor(out=diff[:], in0=hi[:], in1=ho[:],
                                        op=ALU.subtract)
                m = gp.tile([P, P], F32, tag="m")
                nc.scalar.activation(m[:], diff[:], ACTF.Copy, scale=g[:])
                xw = gp.tile([P, P], BF16, tag="xw")
                nc.sync.dma_start(out=xw[:], in_=x_own[rows, :])
                f1 = gp.tile([P, P], F32, tag="f1")
                nc.vector.tensor_add(out=f1[:], in0=m[:], in1=ho[:])
                f2 = gp.tile([P, P], BF16, tag="f2")
                nc.vector.tensor_add(out=f2[:], in0=f1[:], in1=xw[:])
                nc.sync.dma_start(out=out[rows, :], in_=f2[:])

    nc.compile()
    return nc


_CACHE = {}


def kernel(x, edge_index, w_s2d, b_s2d, w_d2s, b_d2s,
           w_e1, b_e1, w_e2, b_e2, w_g1, b_g1, w_g2, b_g2):
    t0 = time.time()
    x = np.asarray(x, np.float32)
    ei = np.asarray(edge_index)
    N, D = x.shape
    per_core = N // NC
    nwin = (per_core + P - 1) // P
    NW = nwin * P
    src = ei[0].astype(np.int64)
    dst = ei[1].astype(np.int64)
    E = src.shape[0]

    w_e1 = np.asarray(w_e1, np.float32)
    w_g1 = np.asarray(w_g1, np.float32)
    xTb = np.ascontiguousarray(x.astype(NPF8).T)          # [128, N]
    W4 = np.concatenate([
        w_e1[:D], np.asarray(w_s2d, np.float32),
        w_e1[D:], np.asarray(w_d2s, np.float32)], 1).astype(NPF8)
    b4 = np.concatenate([
        np.zeros(P, np.float32), np.asarray(b_s2d, np.float32),
        np.asarray(b_e1, np.float32), np.asarray(b_d2s, np.float32),
    ]).reshape(1, 4 * P).astype(NPF8)
    t0 = _t("tables(host)", t0)

    # --- edge routing (int32 throughout: ~2x less memory traffic) ---
    src32 = src.astype(np.int32)
    dst32 = dst.astype(np.int32)
    counts_max = 0
    orders = []
    for d, key in enumerate((dst32, src32)):
        owner = key // per_core
        local = key - owner * per_core
        win = local // P
        sk = (owner * nwin + win).astype(np.int32)
        order = np.argsort(sk, kind="stable").astype(np.int32)
        orders.append((order, owner, local, win))
        cnt = np.bincount(sk, minlength=NC * nwin)
        counts_max = max(counts_max, int(cnt.max()))
    T = max(1, (counts_max + P - 1) // P)
    WT = nwin * T

    metas = []
    for d, key in enumerate((dst32, src32)):
        other = src32 if d == 0 else dst32
        order, owner, local, win = orders[d]
        IDX = np.zeros((NC, P, WT), np.int32)
        JDX = np.zeros((NC, P, WT), np.int32)
        DL = np.full((NC, P, WT), 999.0, NPBF)
        deg = np.bincount(owner * NW + local,
                          minlength=NC * NW).reshape(NC, NW)
        RC = np.zeros((NC, P, nwin), np.float32)
        RC[:, :, :] = (1.0 / np.maximum(deg, 1.0)).reshape(
            NC, nwin, P).transpose(0, 2, 1)
        o_owner = owner[order]
        o_win = win[order]
        o_local = local[order]
        o_other = other[order]
        o_key = key[order]
        flat = (o_owner * nwin + o_win).astype(np.int32)
        start = np.searchsorted(flat, np.arange(NC * nwin, dtype=np.int32)
                                ).astype(np.int32)
        j = np.arange(E, dtype=np.int32) - start[flat]
        p = j % P
        t = j // P
        c_ = o_win * T + t
        fi = o_owner * (P * WT) + p * WT + c_
        IDX.reshape(-1)[fi] = o_other
        JDX.reshape(-1)[fi] = o_key
        DL.reshape(-1)[fi] = (o_local % P).astype(np.float32)
        metas.append((IDX, JDX, DL, RC))
    t0 = _t("routing(host)", t0)

    has_b_g1 = bool(np.any(np.asarray(b_g1) != 0))
    consts = {
        "xT": xTb, "W4": W4, "b4": b4,
        "wg1a": w_g1[:P], "wg1b": w_g1[P:],
        "we2r": np.tile(np.asarray(w_e2, np.float32).reshape(1, P),
                        (P, 1)).astype(NPBF),
        "wg2r": np.tile(np.asarray(w_g2, np.float32).reshape(1, P), (P, 1)),
        "iota": np.tile(np.arange(P, dtype=np.float32), (P, 1)).astype(NPBF),
        "ident": np.eye(P, dtype=np.float32),
        "be2c": np.full((P, 1), float(np.asarray(b_e2).reshape(-1)[0]),
                        np.float32),
        "bg2c": np.full((P, 1), float(np.asarray(b_g2).reshape(-1)[0]),
                        np.float32),
        "ones_row": np.ones((1, P), np.float32),
    }
    if has_b_g1:
        consts["bg1r"] = np.asarray(b_g1, np.float32).reshape(1, P)

    key = (nwin, T, has_b_g1)
    if key not in _CACHE:
        _CACHE[key] = _build(*key)
        t0 = _t("bass-compile", t0)
    nc = _CACHE[key]

    in_maps = []
    (IDX0, JDX0, DL0, RC0), (IDX1, JDX1, DL1, RC1) = metas
    for c in range(NC):
        m = dict(consts)
        m.update({
            "idx0": IDX0[c], "jdx0": JDX0[c], "dl0": DL0[c], "rc0": RC0[c],
            "idx1": IDX1[c], "jdx1": JDX1[c], "dl1": DL1[c], "rc1": RC1[c],
        })
        xo = np.zeros((NW, P), NPBF)
        xo[:per_core] = x[c * per_core:(c + 1) * per_core].astype(NPBF)
        m["x_own"] = xo
        in_maps.append(m)
    t0 = _t("in_maps(host)", t0)

    res = run_bass_kernel_spmd(nc, in_maps, list(range(NC)))
    global LAST_RESULT
    LAST_RESULT = res
    t0 = _t("run+fetch", t0)
    out = np.concatenate(
        [res.results[c]["out"][:per_core].astype(np.float32)
         for c in range(NC)], axis=0)
    return out
